# revision 44
# baseline (speedup 1.0000x reference)
"""Trainium2 Bass kernel for nn_Attention (v11) - bf16, fully SBUF-resident.

Sharding (Megatron-style TP x DP): data-parallel over the 2 batches (cores
0-3 / 4-7), head-parallel (8 heads per core) within each 4-core group.  The
output projection is ROW-parallel: each core multiplies its 8 heads' attention
outputs by its 1024-row slice of wo and emits a bf16 partial [4096, 2048];
the 4 partials per batch are summed in fp32 on the host.  No collectives.

Design (vs the fp32 v6 baseline, ~1.57 ms -> ~1.36 ms measured here):
 - All GEMM operands are bf16 (weights, x, Q/K/V, E, attention outputs).
   PSUM accumulation stays fp32, softmax denominators/normalization fp32.
   Measured end-to-end absmax err / output scale ~4.7e-3 (gate 2e-2).
 - Q/K/V never touch HBM: they are computed into resident SBUF tiles
   (8.4 MiB Q+K bf16, 4.2 MiB V) and consumed by attention directly.
   RoPE outputs are scattered from pair-block layout into per-head
   [64 evens | 64 odds] partition layout by SBUF->SBUF DMA; S = K.Q is
   invariant to that (consistent) dim permutation.
 - Attention S chunks are exp'd in PAIRS ([128, 2, 512] PSUM tiles) to
   halve ACT's fixed per-instruction overhead; causal masking is an
   additive -1e5 bias PRE-FILLED into the diagonal chunks' PSUM (the S
   matmuls accumulate onto it with start=False), so exp emits exact zeros
   and nothing sits between exp and PV on the critical path.
 - Softmax denominator via DVE chunk-accumulation plus one small
   ones-matmul per (head, q-block) for the cross-partition reduction -
   removes the per-chunk ones-matmul pass (~164k PE cycles).
 - The output projection is FUSED into attention as a staircase: after
   q-block ql completes for all heads, its token range's O-chunks are
   interleaved per-head into the next q-block's instruction stream as
   pure-PE fill-in (queued before the pd matmul so the in-order PE queue
   never idles at cross-engine waits).  wo is re-read once per q-block
   (4x, 33 MB bf16 total) to enable this - DMA has ample headroom.
 - Output partials written bf16 (host sums in fp32).

Per-core DMA ~118 MB, fully overlapped under ~1.04 ms of PE matmul work
(sim: 92% PE occupancy; phase A measured on-HW at cost-model parity).
"""
import numpy as np

import concourse.bass as bass
import concourse.mybir as mybir
import concourse.tile as tile
from concourse.bass_utils import run_bass_kernel_spmd

P = 128
DIM = 4096
NH = 32
HD = 128
B = 2
S = 2048
NCORES = 8
NGRP = 4
HPC = NH // NGRP          # 8 heads per core
DPC = HPC * HD            # 1024 dims per core
NPAIR = HPC // 2          # 4 head pairs
TBLK = 1024
QT = 512
KI = DIM // P             # 32
VQ = 512                  # V-projection dv strip width (moving operand)
KS = 4                    # i-chunks per streamed wv tile

F32 = mybir.dt.float32
F32R = mybir.dt.float32r
BF16 = mybir.dt.bfloat16
SCALE = 1.0 / float(np.sqrt(HD))


def _split_excess_waits(nc, max_waits=1):
    """TRN2 TPB instructions embed exactly one sync-wait slot; Tile can emit
    several per instruction and walrus then fails with "Too many sync wait
    commands".  Hoist all but one wait onto EventSemaphore instructions
    inserted before the instruction on the same engine queue."""
    n = 0
    for f in nc.m.functions:
        for b in f.blocks:
            out = []
            changed = False
            for i in b.instructions:
                si = i.sync_info
                if si is not None and len(si.on_wait) > max_waits:
                    waits = list(si.on_wait)
                    extra, keep = waits[:-max_waits], waits[-max_waits:]
                    for k, w in enumerate(extra):
                        es = mybir.InstEventSemaphore(
                            name=f"{i.name}-wsplit{k}", ins=[], outs=[])
                        es.engine = i.engine
                        es.sync_info = mybir.SyncInfo(on_wait=[w], on_update=[])
                        out.append(es)
                        n += 1
                    i.sync_info = mybir.SyncInfo(
                        on_wait=keep, on_update=list(si.on_update))
                    changed = True
                out.append(i)
            if changed:
                b.instructions = out
    return n


def _proj_block(nc, tc, x_re, wqk_re, wv_re, cos2_d, sin2_d, k_sb, q_sb,
                v_sb, tb):
    """One 1024-token block: Q/K per head-pair with full-width RoPE first
    (their i-chunked chains consume x as it streams in), then V (natural
    layout via x-stationary matmuls).  Q/K/V land in resident SBUF tiles."""
    t0 = tb * TBLK
    KH = KI // 2
    with (
        tc.tile_pool(name="xa_p", bufs=1) as xa_p,
        tc.tile_pool(name="wv_p", bufs=2) as wv_p,
    ):
        with (
            tc.tile_pool(name="w_p", bufs=4) as w_p,
            tc.tile_pool(name="cs_p", bufs=2) as cs_p,
            tc.tile_pool(name="rope_p", bufs=2) as rope_p,
            tc.tile_pool(name="qk_p", bufs=3) as qk_p,
            tc.tile_pool(name="psQK", bufs=3, space="PSUM") as psQK,
        ):
            # prefetch pair 0's weights (K pair 0) ahead of the big x fill
            w0a = w_p.tile([P, KH, P], BF16, tag="wt")
            w0b = w_p.tile([P, KH, P], BF16, tag="wt")
            w0oa = w_p.tile([P, KH, P], BF16, tag="wt")
            w0ob = w_p.tile([P, KH, P], BF16, tag="wt")
            nc.scalar.dma_start(w0a[:], wqk_re[:, 0:KH, DPC:DPC + P])
            nc.scalar.dma_start(w0b[:], wqk_re[:, KH:KI, DPC:DPC + P])
            nc.scalar.dma_start(w0oa[:], wqk_re[:, 0:KH, DPC + P:DPC + 2 * P])
            nc.scalar.dma_start(w0ob[:], wqk_re[:, KH:KI, DPC + P:DPC + 2 * P])
            csl = cs_p.tile([P, TBLK], BF16, tag="cs")
            nc.scalar.dma_start(csl[:], cos2_d[:, t0:t0 + TBLK])
            snl = cs_p.tile([P, TBLK], BF16, tag="cs")
            nc.scalar.dma_start(snl[:], sin2_d[:, t0:t0 + TBLK])
            xa = xa_p.tile([P, KI, TBLK], BF16, tag="xa")
            for i in range(KI):
                nc.sync.dma_start(xa[:, i], x_re[:, i, t0:t0 + TBLK])

            for pr in range(2 * NPAIR):
                qk, j = divmod(pr, NPAIR)
                qk = 1 - qk          # K pairs first: attention needs K earliest
                c0 = qk * DPC + j * 256
                if pr == 0:
                    wea, web, woa, wob = w0a, w0b, w0oa, w0ob
                else:
                    wea = w_p.tile([P, KH, P], BF16, tag="wt")
                    web = w_p.tile([P, KH, P], BF16, tag="wt")
                    nc.scalar.dma_start(wea[:], wqk_re[:, 0:KH, c0:c0 + P])
                    nc.scalar.dma_start(web[:], wqk_re[:, KH:KI, c0:c0 + P])
                    woa = w_p.tile([P, KH, P], BF16, tag="wt")
                    wob = w_p.tile([P, KH, P], BF16, tag="wt")
                    nc.scalar.dma_start(woa[:],
                                        wqk_re[:, 0:KH, c0 + P:c0 + 2 * P])
                    nc.scalar.dma_start(wob[:],
                                        wqk_re[:, KH:KI, c0 + P:c0 + 2 * P])
                pe_t = psQK.tile([P, TBLK], F32, tag="pq")
                po_t = psQK.tile([P, TBLK], F32, tag="pq")
                NQH = TBLK // QT
                if pr == 0:
                    # interleave evens/odds chains i-major: both trail the
                    # streaming x fill instead of the odds chain serializing
                    # after it
                    for i in range(KI):
                        wtE = wea if i < KH else web
                        wtO = woa if i < KH else wob
                        for dst, wt in ((pe_t, wtE), (po_t, wtO)):
                            for th in range(NQH):
                                nc.tensor.matmul(
                                    dst[:, th * QT:(th + 1) * QT],
                                    wt[:, i % KH],
                                    xa[:, i, th * QT:(th + 1) * QT],
                                    start=(i == 0), stop=(i == KI - 1),
                                    skip_group_check=True)
                else:
                    for dst, wA, wB in ((pe_t, wea, web), (po_t, woa, wob)):
                        for i in range(KI):
                            wt = wA if i < KH else wB
                            for th in range(NQH):
                                nc.tensor.matmul(
                                    dst[:, th * QT:(th + 1) * QT],
                                    wt[:, i % KH],
                                    xa[:, i, th * QT:(th + 1) * QT],
                                    start=(i == 0), stop=(i == KI - 1))
                qe = qk_p.tile([P, TBLK], BF16, tag="qk")
                qo = qk_p.tile([P, TBLK], BF16, tag="qk")
                for th in range(TBLK // QT):
                    sl = slice(th * QT, (th + 1) * QT)
                    cst = csl[:, th * QT:(th + 1) * QT]
                    snt = snl[:, th * QT:(th + 1) * QT]
                    t1 = rope_p.tile([P, QT], F32, tag="tA")
                    t2 = rope_p.tile([P, QT], F32, tag="tB")
                    nc.vector.tensor_tensor(t1[:], pe_t[:, sl], cst,
                                            mybir.AluOpType.mult)
                    nc.vector.tensor_tensor(t2[:], po_t[:, sl], snt,
                                            mybir.AluOpType.mult)
                    nc.vector.tensor_tensor(qe[:, sl], t1[:], t2[:],
                                            mybir.AluOpType.subtract)
                    t3 = rope_p.tile([P, QT], F32, tag="tA")
                    t4 = rope_p.tile([P, QT], F32, tag="tB")
                    nc.vector.tensor_tensor(t3[:], pe_t[:, sl], snt,
                                            mybir.AluOpType.mult)
                    nc.vector.tensor_tensor(t4[:], po_t[:, sl], cst,
                                            mybir.AluOpType.mult)
                    nc.vector.tensor_tensor(qo[:, sl], t3[:], t4[:],
                                            mybir.AluOpType.add)
                # scatter pair-block [ev h2j|ev h2j+1] / [od ...] into the
                # per-head [64 ev | 64 od] resident layout (SBUF->SBUF)
                dst = k_sb if qk == 1 else q_sb
                nc.gpsimd.dma_start(dst[0:64, 2 * j, t0:t0 + TBLK], qe[0:64])
                nc.gpsimd.dma_start(dst[64:128, 2 * j, t0:t0 + TBLK],
                                    qo[0:64])
                nc.gpsimd.dma_start(dst[0:64, 2 * j + 1, t0:t0 + TBLK],
                                    qe[64:128])
                nc.gpsimd.dma_start(dst[64:128, 2 * j + 1, t0:t0 + TBLK],
                                    qo[64:128])

        NTB = TBLK // P
        with (
            tc.tile_pool(name="psV", bufs=1, space="PSUM") as psV,
        ):
            for half in range(DPC // VQ):
                pvs = [psV.tile([P, VQ], F32, tag=f"pv{tt}",
                                name=f"pv{tb}_{half}_{tt}")
                       for tt in range(NTB)]
                for q in range(KI // KS):
                    wq_t = wv_p.tile([P, KS, VQ], BF16, tag="wv",
                                     name=f"wv{tb}_{half}_{q}")
                    nc.sync.dma_start(
                        wq_t[:], wv_re[:, q * KS:(q + 1) * KS,
                                       half * VQ:(half + 1) * VQ])
                    for ik in range(KS):
                        i = q * KS + ik
                        for tt in range(NTB):
                            nc.tensor.matmul(pvs[tt][:],
                                             xa[:, i, tt * P:(tt + 1) * P],
                                             wq_t[:, ik],
                                             start=(i == 0),
                                             stop=(i == KI - 1),
                                             skip_group_check=True)
                            if i == KI - 1:
                                # copy right after each psum's final matmul:
                                # copies overlap the remaining tt's chains
                                dst = v_sb[:, tb * NTB + tt,
                                           half * VQ:(half + 1) * VQ]
                                if tt % 2 == 0:
                                    nc.vector.tensor_copy(dst, pvs[tt][:])
                                else:
                                    nc.scalar.activation(
                                        dst, pvs[tt][:],
                                        mybir.ActivationFunctionType.Copy)


def _attn_oproj(nc, tc, k_sb, q_sb, v_sb, ones, cmask, attn_sb, wo_re, outT):
    """Causal attention from resident SBUF Q/K/V fused with the output
    projection.  S chunks are exp'd in PAIRS ([128, 2, 512] PSUM tiles) to
    halve ACT's fixed per-instruction overhead, and after each q-block ql
    completes for all heads, the O-projection pass for that token range is
    emitted - its pure-PE matmuls fill attention's ACT-bound gaps so the PE
    never idles (and never HAM-downclocks) during this phase."""
    with (
        tc.tile_pool(name="es_p", bufs=4) as es_p,
        tc.tile_pool(name="dn_p", bufs=3) as dn_p,
        tc.tile_pool(name="at_p", bufs=3) as at_p,
        tc.tile_pool(name="wo_p", bufs=8) as wo_p,
        tc.tile_pool(name="o_p", bufs=4) as o_p,
        tc.tile_pool(name="psS", bufs=2, space="PSUM") as psS,
        tc.tile_pool(name="psD", bufs=1, space="PSUM") as psD,
        tc.tile_pool(name="psO", bufs=1, space="PSUM") as psO,
        tc.tile_pool(name="psE", bufs=2, space="PSUM") as psE,
    ):
        def o_chunk(ts, oc, wt):
            """One output-projection chunk [128 out-dims, 512 tokens]."""
            poE = psE.tile([P, QT], F32, tag="poE", name=f"poE{ts}_{oc}")
            for dvc in range(HPC):
                nc.tensor.matmul(poE[:], wt[:, dvc],
                                 attn_sb[:, dvc, ts * QT:(ts + 1) * QT],
                                 start=(dvc == 0), stop=(dvc == HPC - 1))
            ob = o_p.tile([P, QT], BF16, tag="ob", name=f"ob{ts}_{oc}")
            if oc % 2 == 0:
                nc.vector.tensor_copy(ob[:], poE[:])
            else:
                nc.scalar.activation(ob[:], poE[:],
                                     mybir.ActivationFunctionType.Copy)
            nc.sync.dma_start(outT[oc * P:(oc + 1) * P,
                                    ts * QT:(ts + 1) * QT], ob[:])

        def o_load(ts, oc):
            wt = wo_p.tile([P, HPC, P], BF16, tag="wo", name=f"wo{ts}_{oc}")
            nc.sync.dma_start(wt[:], wo_re[:, :, oc * P:(oc + 1) * P])
            return wt

        OPH = (DIM // P) // HPC   # 4 O-chunks interleaved per head
        for ql in range(S // QT):
            for h in range(HPC):
                # prefetch this head's O-chunk weights (consumed after the
                # head's normalize as PE fill-in work)
                if ql >= 1:
                    wts = [o_load(ql - 1, OPH * h + i) for i in range(OPH)]
                q0 = ql * QT
                nk = (q0 + QT) // P
                kd = q0 // P          # first diagonal chunk
                po = psO.tile([P, QT], F32, tag="po")
                da2 = dn_p.tile([P, 2, QT], F32R, tag="dacc")
                es_list = []

                def s_stage(m):
                    # chunk pair (2m, 2m+1): two S matmuls, one wide exp.
                    # Diagonal chunks get a -1e5 additive mask PRE-FILLED
                    # into PSUM (start=False accumulate onto it): exp then
                    # emits exact zeros and nothing sits between exp and PV
                    # on the critical path.
                    ps2 = psS.tile([P, 2, QT], F32, tag="ps",
                                   name=f"ps{h}_{ql}_{m}")
                    diag0 = max(0, kd - 2 * m)
                    if 2 * m + 1 >= kd:
                        nc.vector.tensor_copy(
                            ps2[:, diag0:2],
                            cmask[:, 2 * m + diag0 - kd:2 * m + 2 - kd, :])
                    for u in range(2):
                        kc = 2 * m + u
                        nc.tensor.matmul(ps2[:, u],
                                         k_sb[:, h, kc * P:(kc + 1) * P],
                                         q_sb[:, h, q0:q0 + QT],
                                         start=(kc < kd), stop=True)
                    es2 = es_p.tile([P, 2, QT], BF16, tag="es",
                                    name=f"es{h}_{ql}_{m}")
                    nc.scalar.activation(es2[:], ps2[:],
                                         mybir.ActivationFunctionType.Exp,
                                         scale=SCALE)
                    if m == 0:
                        nc.vector.tensor_copy(da2[:], es2[:])
                    else:
                        nc.vector.tensor_tensor(da2[:], da2[:], es2[:],
                                                mybir.AluOpType.add)
                    es_list.append(es2)

                def pv_stage(m):
                    for u in range(2):
                        kc = 2 * m + u
                        nc.tensor.matmul(po[:],
                                         v_sb[:, kc, h * HD:(h + 1) * HD],
                                         es_list[m][:, u],
                                         start=(kc == 0), stop=(kc == nk - 1),
                                         skip_group_check=True)

                nm = nk // 2
                DEPTH = 1        # keep PE one exp-pair ahead of ACT
                for m in range(nm):
                    s_stage(m)
                    if m >= DEPTH:
                        pv_stage(m - DEPTH)
                for m in range(max(0, nm - DEPTH), nm):
                    pv_stage(m)

                dacc = dn_p.tile([P, QT], F32R, tag="dfold")
                nc.vector.tensor_tensor(dacc[:], da2[:, 0], da2[:, 1],
                                        mybir.AluOpType.add)
                # O-chunks for the previous q-block: queued BEFORE the pd
                # matmul (the in-order PE queue would otherwise idle at pd's
                # wait on the DVE fold) - pure-PE fill-in while the
                # normalize chain drains
                if ql >= 1:
                    for i in range(OPH):
                        o_chunk(ql - 1, OPH * h + i, wts[i])
                pd = psD.tile([P, QT], F32, tag="pd")
                nc.tensor.matmul(pd[:], ones[:], dacc[:],
                                 start=True, stop=True)
                rcp = at_p.tile([P, QT], F32, tag="rcp")
                nc.vector.reciprocal(rcp[:], pd[:])
                nc.vector.tensor_tensor(attn_sb[:, h, q0:q0 + QT], po[:],
                                        rcp[:], mybir.AluOpType.mult)

        # final token range: all heads done, run its full O pass
        ts = S // QT - 1
        wts = [o_load(ts, oc) for oc in range(4)]
        for oc in range(DIM // P):
            if oc + 4 < DIM // P:
                wts.append(o_load(ts, oc + 4))
            o_chunk(ts, oc, wts[oc])


def build_nc(reps=1, phases="ABD"):
    nc = bass.Bass(trn_type="TRN2", num_devices=NCORES)

    xT = nc.dram_tensor("xT", [DIM, S], BF16, kind="ExternalInput")
    wqkT = nc.dram_tensor("wqkT", [DIM, 2 * DPC], BF16, kind="ExternalInput")
    wvT = nc.dram_tensor("wvT", [DIM, DPC], BF16, kind="ExternalInput")
    woT = nc.dram_tensor("woT", [DPC, DIM], BF16, kind="ExternalInput")
    cos2_d = nc.dram_tensor("cos2_d", [P, S], BF16, kind="ExternalInput")
    sin2_d = nc.dram_tensor("sin2_d", [P, S], BF16, kind="ExternalInput")
    ones_d = nc.dram_tensor("ones_d", [P, P], F32R, kind="ExternalInput")
    cmask_d = nc.dram_tensor("cmask_d", [P, 4, QT], BF16,
                             kind="ExternalInput")

    outT = nc.dram_tensor("outT", [DIM, S], BF16, kind="ExternalOutput")

    x_re = xT.rearrange("(io p) t -> p io t", p=P)
    wqk_re = wqkT.rearrange("(io p) c -> p io c", p=P)
    wv_re = wvT.rearrange("(io p) c -> p io c", p=P)
    wo_re = woT.rearrange("(c p) o -> p c o", p=P)

    with tile.TileContext(nc) as tc:
        with tc.tile_pool(name="const", bufs=1) as const:
            ones = const.tile([P, P], F32R)
            nc.scalar.dma_start(ones[:], ones_d[:])
            cmask = const.tile([P, 4, QT], BF16, tag="cmask")
            nc.gpsimd.dma_start(cmask[:], cmask_d[:])

            for _rep in range(reps):
                with tc.tile_pool(name="kqv_p", bufs=1) as kqv_p:
                    k_sb = kqv_p.tile([P, HPC, S], BF16, tag="ksb",
                                      name=f"ksb{_rep}")
                    q_sb = kqv_p.tile([P, HPC, S], BF16, tag="qsb",
                                      name=f"qsb{_rep}")
                    v_sb = kqv_p.tile([P, S // P, DPC], BF16, tag="vsb",
                                      name=f"vsb{_rep}")
                    if "A" in phases:
                        for tb in range(S // TBLK):
                            _proj_block(nc, tc, x_re, wqk_re, wv_re,
                                        cos2_d, sin2_d, k_sb, q_sb,
                                        v_sb, tb)
                    with tc.tile_pool(name="attn_p", bufs=1) as attn_p:
                        attn_sb = attn_p.tile([P, HPC, S], BF16, tag="attn",
                                              name=f"attn{_rep}")
                        if "B" in phases:
                            _attn_oproj(nc, tc, k_sb, q_sb, v_sb, ones,
                                        cmask, attn_sb, wo_re, outT)

    _split_excess_waits(nc)
    return nc


_NC_CACHE = None


def _get_nc():
    global _NC_CACHE
    if _NC_CACHE is None:
        _NC_CACHE = build_nc()
    return _NC_CACHE


def _pair_perm(w):
    """[1024, in] head-major rows -> head-pair blocks [128 evens | 128 odds]."""
    w4 = w.reshape(NPAIR, 2, HD, DIM)
    ev = w4[:, :, 0::2, :]
    od = w4[:, :, 1::2, :]
    return np.stack([ev, od], axis=1).reshape(DPC, DIM)


def make_in_maps(x, wqk_w, wv_w, wo_w):
    """Build the 8 per-core input dicts."""
    import ml_dtypes
    bf = ml_dtypes.bfloat16
    x = np.asarray(x, dtype=np.float32)
    wqk_w = np.asarray(wqk_w, dtype=np.float32)
    wv_w = np.asarray(wv_w, dtype=np.float32)
    wo_w = np.asarray(wo_w, dtype=np.float32)

    inv_freq = (1.0 / (10000.0 ** (np.arange(0, HD, 2, dtype=np.float32)
                                   / np.float32(HD)))).astype(np.float32)
    tpos = np.arange(S, dtype=np.float32)
    freqs = tpos[:, None] * inv_freq[None, :]          # [S, 64]
    cos2 = np.ascontiguousarray(
        np.tile(np.cos(freqs).T, (2, 1)).astype(bf))   # [128, S]
    sin2 = np.ascontiguousarray(
        np.tile(np.sin(freqs).T, (2, 1)).astype(bf))

    ones = np.ones((P, P), dtype=np.float32)
    kk = np.arange(P)[:, None]
    qq = np.arange(QT)[None, :]
    allowed = np.stack([(qq - kk - P * j >= 0) for j in range(4)], axis=1)
    cmask = np.ascontiguousarray(
        np.where(allowed, 0.0, -1e5).astype(bf))   # additive pre-exp mask

    xT_b = [np.ascontiguousarray(x[b].T.astype(bf)) for b in range(B)]

    in_maps = []
    for core in range(NCORES):
        b, g = divmod(core, NGRP)
        r0 = g * DPC
        wq = _pair_perm(wqk_w[r0:r0 + DPC])
        wk = _pair_perm(wqk_w[DIM + r0:DIM + r0 + DPC])
        wqkT_c = np.ascontiguousarray(
            np.concatenate([wq, wk], axis=0).T.astype(bf))
        wvT_c = np.ascontiguousarray(wv_w[r0:r0 + DPC].T.astype(bf))
        woT_c = np.ascontiguousarray(wo_w[:, r0:r0 + DPC].T.astype(bf))
        in_maps.append({
            "xT": xT_b[b],
            "wqkT": wqkT_c,
            "wvT": wvT_c,
            "woT": woT_c,
            "cos2_d": cos2,
            "sin2_d": sin2,
            "ones_d": ones,
            "cmask_d": cmask,
        })
    return in_maps


def assemble(results):
    out = np.empty((B, S, DIM), dtype=np.float32)
    for b in range(B):
        acc = results[b * NGRP]["outT"].astype(np.float32)
        for g in range(1, NGRP):
            acc += results[b * NGRP + g]["outT"].astype(np.float32)
        out[b] = acc.T
    return out


def kernel(x, wqk_w, wv_w, wo_w):
    nc = _get_nc()
    in_maps = make_in_maps(x, wqk_w, wv_w, wo_w)
    res = run_bass_kernel_spmd(nc, in_maps, core_ids=list(range(NCORES)))
    return assemble(res.results)
